# revision 19
# baseline (speedup 1.0000x reference)
"""BEiT attention block on 8 TRN2 NeuronCores, data-parallel over batch.

Full inputs -> kernel(**inputs) -> full output (16, 1025, 768) f32.

Per-core work: 2 batches of multi-head attention (N=1025 tokens, C=768,
H=12 heads, d=64) with a relative-position bias added to the logits.

Strategy (all matmul operands bf16, f32 PSUM accumulation):
  - host: transpose x -> xT (C, N), pre-transpose weights, fold the
    softmax into exp(s*scale) * exp(bias) with exp(bias^T) precomputed
    in bf16 (padded kpos rows are 0 so padded keys vanish from sums).
  - device per batch: qkvT = Wqk^T.T @ xT (q,k kept d-on-partitions),
    v in natural token-on-partitions layout with a ones column appended
    (PV then yields both attn@v and the softmax denominators).
  - scores computed transposed: sT[k, q] = k_h.T @ q_h, contraction d=64;
    the two heads of a pair sit at partitions 0-63 / 64-127 so their
    QK^T matmuls row-tile the PE array concurrently.
  - softmax denominators collected into one [24, N] tile (cross-partition
    via DMA), one batched DVE reciprocal, broadcast back via DMA with a
    partition-step-0 access pattern, applied to unnormalized outputs.
  - proj: y = a^T.T @ Wp^T + b, a^T already in the right layout.
"""

import numpy as np
import ml_dtypes

B = 16
N = 1025
C = 768
H = 12
D = 64
NCORES = 8
BPC = B // NCORES  # batches per core
NPAD = 1152        # padded key length: 9 * 128
KB = NPAD // 128   # key blocks
IB = C // 128      # input-channel blocks
NQ = 1026          # query extent incl. one even-ing pad column
QCS = [384, 384, 258]   # query chunks covering NQ (all even -> DVE 2x mode)
QCO = [0, 384, 768]
SCALE = D ** -0.5
# minimax cubic p(z) ~ exp(z) on [-0.55, 0.55], factored a3*(z-R)*(z^2+S*z+T);
# exp(y)*EB computed as (p(y/4) * a3*EB^(1/4))^4 in one custom DVE op.
EXP_R = -1.658048394110858
EXP_S = 1.462861309003841
EXP_T = 3.672443055287797
EXP_A3 = 0.1641584267735188
BF16 = ml_dtypes.bfloat16

_cache: dict = {}


def _register_exp4():
    """Register the fused quartic-exp custom DVE op: out = (p(z)*Src1)^4
    with p monic-factored; Src1 carries a3*EB^(1/4)."""
    if "exp4" in _cache:
        return _cache["exp4"]
    import numpy as np
    from concourse import dve_ops
    from concourse.dve_spec import Spec, Src0, Src1, C0, C1, C2
    from concourse.dve_table_gen import dve_ver_for
    from concourse.dve_uop import DveOpSpec

    name = "EXP4_EB_ANT"
    for op in dve_ops.OPS:
        if op.name == name:
            _cache["exp4"] = op
            return op

    m5 = (((Src0 + C1) * Src0) + C2) * (Src0 - C0) * Src1
    p2 = m5 * m5
    body = p2 * p2

    def ref(in0, in1, s0, s1, imm2):
        m5 = ((((in0 + s1) * in0) + imm2) * (in0 - s0) * in1).astype(np.float32)
        p2 = (m5 * m5).astype(np.float32)
        return (p2 * p2).astype(np.float32)

    op = dve_ops.DveOp(name, Spec(body=body, reference=ref), subdim=False,
                       uops_sha={})
    row = dve_ops._CUSTOM_DVE_ROW_BASE + len(dve_ops.OPS)
    assert row < 0x20
    dve_ops.OPS.append(op)
    dve_ops._SUB_OPCODE_FOR_NAME[name] = row
    dve_ops.CUSTOM_DVE_SPECS[name] = op.spec
    ver = dve_ver_for("TRN2")
    spec = DveOpSpec(name=name, opcode=row,
                     uops=dve_ops.lower(op.spec, ver=ver),
                     rd1_en=dve_ops.has_src1(op.spec))
    op.uops_sha[ver] = spec.sha(ver)
    _cache["exp4"] = op
    return op


def _build():
    import concourse.bass as bass
    import concourse.mybir as mybir
    import concourse.tile as tile
    from concourse import bacc

    dt = mybir.dt
    f32 = dt.float32
    bf = dt.bfloat16
    AFT = mybir.ActivationFunctionType

    exp4_op = _register_exp4()
    nc = bacc.Bacc("TRN2", target_bir_lowering=False, debug=False)

    xT_d = nc.declare_dram_parameter("xT", [BPC, C, NPAD], bf, isOutput=False)
    wqk_d = nc.declare_dram_parameter("wqk", [C, 2 * C], bf, isOutput=False)
    wv_d = nc.declare_dram_parameter("wv", [C, C], bf, isOutput=False)
    wp_d = nc.declare_dram_parameter("wp", [C, C], bf, isOutput=False)
    qkb_d = nc.declare_dram_parameter("qkb", [128, 2 * IB], f32, isOutput=False)
    vb_d = nc.declare_dram_parameter("vb", [128, C], f32, isOutput=False)
    pb_d = nc.declare_dram_parameter("pb", [128, C], f32, isOutput=False)
    eb_d = nc.declare_dram_parameter("eb", [H, 3, NPAD, 384], bf, isOutput=False)
    out_d = nc.declare_dram_parameter("out", [BPC, N, C], f32, isOutput=True)

    with tile.TileContext(nc) as tc:
        from contextlib import ExitStack

        ctx = ExitStack()
        with ctx:
            consts = ctx.enter_context(tc.tile_pool(name="consts", bufs=1))
            persist = ctx.enter_context(tc.tile_pool(name="persist", bufs=1))

            # ---- constants ----
            wqk_sb = consts.tile([128, IB, 2 * C], bf)
            nc.sync.dma_start(wqk_sb[:], wqk_d.ap().rearrange("(ib p) o -> p ib o", p=128))
            wp_sb = consts.tile([128, IB, C], bf)
            nc.sync.dma_start(wp_sb[:], wp_d.ap().rearrange("(ib p) o -> p ib o", p=128))
            qkb_sb = consts.tile([128, 2 * IB], f32)
            nc.sync.dma_start(qkb_sb[:], qkb_d.ap())
            vb_sb = consts.tile([128, C], f32)
            nc.sync.dma_start(vb_sb[:], vb_d.ap())
            pb_sb = consts.tile([128, C], f32)
            nc.sync.dma_start(pb_sb[:], pb_d.ap())

            # ---- persistent per-batch tensors ----
            xT_sb = []
            v_sb = []
            a_sb = []
            for b in range(BPC):
                t = persist.tile([128, IB, NPAD], bf, name=f"xT{b}")
                nc.sync.dma_start(t[:], xT_d.ap()[b].rearrange("(ib p) q -> p ib q", p=128))
                xT_sb.append(t)
                v_sb.append(persist.tile([128, KB, H, D + 1], bf, name=f"v{b}"))
                a_sb.append(persist.tile([128, IB, NPAD], bf, name=f"a{b}"))
                # zero the padded query columns so proj sees no garbage
                nc.gpsimd.memset(a_sb[b][:, :, NQ:], 0.0)

            sums_sb = [persist.tile([H, NQ], f32, name=f"sums{b}")
                       for b in range(BPC)]

            # ---- v projection (natural layout, + ones column) ----
            with tc.tile_pool(name="vproj_ps", bufs=2, space="PSUM") as vps, \
                 tc.tile_pool(name="wv_pool", bufs=1) as wvp:
                wv_sb = wvp.tile([128, IB, C], bf)
                nc.sync.dma_start(wv_sb[:], wv_d.ap().rearrange("(ib p) o -> p ib o", p=128))
                for b in range(BPC):
                    nc.gpsimd.memset(v_sb[b][:, :, :, D:], 1.0)
                    for kpb in range(KB):
                        for vc in range(2):
                            ps = vps.tile([128, 384], f32, name="vps")
                            for ib in range(IB):
                                nc.tensor.matmul(
                                    ps[:],
                                    lhsT=xT_sb[b][:, ib, kpb * 128:(kpb + 1) * 128],
                                    rhs=wv_sb[:, ib, vc * 384:(vc + 1) * 384],
                                    start=(ib == 0),
                                    stop=(ib == IB - 1),
                                )
                            nc.vector.tensor_add(
                                out=v_sb[b][:, kpb, 6 * vc:6 * (vc + 1), :D],
                                in0=ps.rearrange("p (h d) -> p h d", d=D),
                                in1=vb_sb[:, vc * 384:(vc + 1) * 384].rearrange(
                                    "p (h d) -> p h d", d=D),
                            )

            # ---- attention over head pairs ----
            attn_ctx = ctx.enter_context(ExitStack())
            qk_pool = attn_ctx.enter_context(tc.tile_pool(name="qk", bufs=4))
            eb_pool = attn_ctx.enter_context(tc.tile_pool(name="ebp", bufs=4))
            ex_pool = attn_ctx.enter_context(tc.tile_pool(name="exp", bufs=3))
            et_pool = attn_ctx.enter_context(tc.tile_pool(name="etmp", bufs=4))
            qk_ps = attn_ctx.enter_context(tc.tile_pool(name="qk_ps", bufs=2, space="PSUM"))
            s_ps = attn_ctx.enter_context(tc.tile_pool(name="s_ps", bufs=4, space="PSUM"))
            o_ps = attn_ctx.enter_context(tc.tile_pool(name="o_ps", bufs=2, space="PSUM"))

            for hp in range(H // 2):
                # q/k projection for this head pair, both batches
                qk2 = []
                for b in range(BPC):
                    t = qk_pool.tile([128, 2, NPAD], bf, name="qk2")
                    nc.gpsimd.memset(t[:, :, N:], 0.0)
                    for sec in range(2):  # 0 = q rows, 1 = k rows
                        ocb = sec * IB + hp
                        for qc in range(3):
                            ps = qk_ps.tile([128, 384], f32, name="qkps")
                            for ib in range(IB):
                                nc.tensor.matmul(
                                    ps[:, :QCS[qc]],
                                    lhsT=wqk_sb[:, ib, sec * C + hp * 128:
                                                sec * C + (hp + 1) * 128],
                                    rhs=xT_sb[b][:, ib, QCO[qc]:QCO[qc] + QCS[qc]],
                                    start=(ib == 0),
                                    stop=(ib == IB - 1),
                                )
                            nc.scalar.activation(
                                out=t[:, sec, QCO[qc]:QCO[qc] + QCS[qc]],
                                in_=ps[:, :QCS[qc]],
                                func=AFT.Identity,
                                bias=qkb_sb[:, ocb:ocb + 1],
                                scale=(SCALE / 4 if sec == 0 else 1.0),
                            )
                    qk2.append(t)

                for qc in range(3):
                    qcs, qco = QCS[qc], QCO[qc]
                    ebt = []
                    for par in range(2):
                        h = 2 * hp + par
                        t = eb_pool.tile([128, KB, 384], bf, name="ebt")
                        nc.sync.dma_start(
                            t[:, :, :qcs],
                            eb_d.ap()[h][qc][:, :qcs].rearrange(
                                "(kb p) q -> p kb q", p=128),
                        )
                        ebt.append(t)
                    for b in range(BPC):
                        ex = [ex_pool.tile([128, KB, 384], bf, name="ex")
                              for _ in range(2)]
                        for kb in range(KB):
                            for par in range(2):
                                p0 = par * 64
                                st = s_ps.tile([128, 384], f32, name="st")
                                nc.tensor.matmul(
                                    st[:, :qcs],
                                    lhsT=qk2[b][p0:p0 + 64, 1, kb * 128:(kb + 1) * 128],
                                    rhs=qk2[b][p0:p0 + 64, 0, qco:qco + qcs],
                                )
                                if par == 0:
                                    nc.vector._custom_dve(
                                        exp4_op,
                                        out=ex[par][:, kb, :qcs],
                                        in0=st[:, :qcs],
                                        in1=ebt[par][:, kb, :qcs],
                                        s0=EXP_R, s1=EXP_S, imm2=EXP_T,
                                    )
                                else:
                                    et = et_pool.tile([128, 384], bf, name="et")
                                    nc.scalar.activation(
                                        out=et[:, :qcs], in_=st[:, :qcs],
                                        func=AFT.Exp, scale=4.0,
                                    )
                                    eng = nc.gpsimd if kb < 7 else nc.vector
                                    eng.tensor_mul(
                                        out=ex[par][:, kb, :qcs],
                                        in0=et[:, :qcs],
                                        in1=ebt[par][:, kb, :qcs],
                                    )
                        for par in range(2):
                            h = 2 * hp + par
                            po = o_ps.tile([D + 1, 384], f32, name="po")
                            for kb in range(KB):
                                nc.tensor.matmul(
                                    po[:, :qcs],
                                    lhsT=v_sb[b][:, kb, h, :],
                                    rhs=ex[par][:, kb, :qcs],
                                    start=(kb == 0),
                                    stop=(kb == KB - 1),
                                )
                            stg = et_pool.tile([65, 384], f32, name="stg")
                            nc.vector.tensor_copy(
                                out=stg[64:65, :qcs], in_=po[D:D + 1, :qcs])
                            nc.sync.dma_start(
                                sums_sb[b][h:h + 1, qco:qco + qcs],
                                stg[64:65, :qcs],
                            )
                            nc.scalar.activation(
                                out=a_sb[b][par * 64:(par + 1) * 64, hp,
                                            qco:qco + qcs],
                                in_=po[:D, :qcs],
                                func=AFT.Copy,
                            )

            attn_ctx.close()

            # ---- per batch: reciprocal, broadcast, normalize, project ----
            recip_dram = nc.dram_tensor("recip_dram", [BPC * H, NQ], bf)
            rb_pool = ctx.enter_context(tc.tile_pool(name="rb", bufs=2))
            with tc.tile_pool(name="y_ps", bufs=4, space="PSUM") as yps, \
                 tc.tile_pool(name="y_sb", bufs=3) as ysb:
                for b in range(BPC):
                    recip_b = persist.tile([H, NQ], bf, name=f"recb{b}")
                    nc.vector.reciprocal_approx_fast(
                        out=sums_sb[b][:], in_=sums_sb[b][:])
                    nc.vector.tensor_copy(out=recip_b[:], in_=sums_sb[b][:])
                    nc.sync.dma_start(
                        recip_dram.ap()[b * H:(b + 1) * H, :], recip_b[:])
                    for h in range(H):
                        p0 = (h % 2) * 64
                        rb = rb_pool.tile([128, NQ], bf, name="rb")
                        src = recip_dram.ap()[b * H + h:b * H + h + 1, :]
                        bcast = bass.AP(
                            tensor=src.tensor,
                            offset=src.offset,
                            ap=[[0, 64]] + list(src.ap[1:]),
                        )
                        nc.sync.dma_start(rb[p0:p0 + 64, :], bcast)
                        nc.vector.tensor_mul(
                            out=a_sb[b][p0:p0 + 64, h // 2, :NQ],
                            in0=a_sb[b][p0:p0 + 64, h // 2, :NQ],
                            in1=rb[p0:p0 + 64, :],
                        )
                    for qb in range(KB):
                        rows = min(128, N - qb * 128)
                        if rows <= 0:
                            continue
                        yt = ysb.tile([128, C], f32, name="yt")
                        for oc2 in range(2):
                            ps = yps.tile([128, 384], f32, name="yps")
                            for ib in range(IB):
                                nc.tensor.matmul(
                                    ps[:],
                                    lhsT=a_sb[b][:, ib, qb * 128:(qb + 1) * 128],
                                    rhs=wp_sb[:, ib, oc2 * 384:(oc2 + 1) * 384],
                                    start=(ib == 0),
                                    stop=(ib == IB - 1),
                                )
                            nc.vector.tensor_add(
                                out=yt[:, oc2 * 384:(oc2 + 1) * 384],
                                in0=ps[:],
                                in1=pb_sb[:, oc2 * 384:(oc2 + 1) * 384],
                            )
                        nc.sync.dma_start(
                            out_d.ap()[b][qb * 128:qb * 128 + rows, :],
                            yt[:rows, :],
                        )

    nc.compile()
    return nc


def _prepare_inputs(x, qkv_weight, q_bias, v_bias, rel_pos_table, proj_weight,
                    proj_bias, rel_pos_index):
    x = np.asarray(x, np.float32)
    qkv_weight = np.asarray(qkv_weight, np.float32)
    q_bias = np.asarray(q_bias, np.float32)
    v_bias = np.asarray(v_bias, np.float32)
    rel_pos_table = np.asarray(rel_pos_table, np.float32)
    proj_weight = np.asarray(proj_weight, np.float32)
    proj_bias = np.asarray(proj_bias, np.float32)
    rel_pos_index = np.asarray(rel_pos_index)

    wqk = np.ascontiguousarray(qkv_weight[:2 * C].T).astype(BF16)
    wv = np.ascontiguousarray(qkv_weight[2 * C:].T).astype(BF16)
    wp = np.ascontiguousarray(proj_weight.T).astype(BF16)

    qkb = np.concatenate([q_bias * np.float32(SCALE / 4), np.zeros(C, np.float32)])
    qkb = np.ascontiguousarray(qkb.reshape(2 * IB, 128).T)  # [128, 12]
    vb = np.ascontiguousarray(np.broadcast_to(v_bias, (128, C)))
    pb = np.ascontiguousarray(np.broadcast_to(proj_bias, (128, C)))

    # exp of transposed rel-pos bias, padded key rows = 0
    bias_qkh = rel_pos_table[rel_pos_index.reshape(-1)].reshape(N, N, H)
    bT = np.zeros((H, N, NQ), np.float64)
    bT[:, :, :N] = bias_qkh.transpose(2, 1, 0)
    ebt = np.zeros((H, NPAD, NQ), BF16)
    for h in range(H):
        if h % 2 == 0:
            ebt[h, :N, :] = (EXP_A3 * np.exp(bT[h] / 4)).astype(BF16)
        else:
            ebt[h, :N, :] = np.exp(bT[h]).astype(BF16)
    ebc = np.zeros((H, 3, NPAD, 384), BF16)
    for qc in range(3):
        ebc[:, qc, :, :QCS[qc]] = ebt[:, :, QCO[qc]:QCO[qc] + QCS[qc]]
    ebt = ebc

    in_maps = []
    for core in range(NCORES):
        xb = x[core * BPC:(core + 1) * BPC]
        xT = np.zeros((BPC, C, NPAD), BF16)
        xT[:, :, :N] = xb.transpose(0, 2, 1).astype(BF16)
        in_maps.append({
            "xT": xT, "wqk": wqk, "wv": wv, "wp": wp,
            "qkb": qkb, "vb": vb, "pb": pb, "eb": ebt,
        })
    return in_maps


def kernel(**inputs) -> np.ndarray:
    from concourse.bass_utils import run_bass_kernel_spmd

    if "nc" not in _cache:
        _cache["nc"] = _build()
    nc = _cache["nc"]

    in_maps = _prepare_inputs(**inputs)
    trace = bool(_cache.get("trace", False))
    res = run_bass_kernel_spmd(nc, in_maps, core_ids=list(range(NCORES)),
                               trace=trace)
    _cache["last_results"] = res
    out = np.concatenate([r["out"] for r in res.results], axis=0)
    return out.astype(np.float32)


# revision 20
# speedup vs baseline: 1.0153x; 1.0153x over previous
"""BEiT attention block on 8 TRN2 NeuronCores, data-parallel over batch.

Full inputs -> kernel(**inputs) -> full output (16, 1025, 768) f32.

Per-core work: 2 batches of multi-head attention (N=1025 tokens, C=768,
H=12 heads, d=64) with a relative-position bias added to the logits.

Strategy (all matmul operands bf16, f32 PSUM accumulation):
  - host: transpose x -> xT (C, N), pre-transpose weights, fold the
    softmax into exp(s*scale) * exp(bias) with exp(bias^T) precomputed
    in bf16 (padded kpos rows are 0 so padded keys vanish from sums).
  - device per batch: qkvT = Wqk^T.T @ xT (q,k kept d-on-partitions),
    v in natural token-on-partitions layout with a ones column appended
    (PV then yields both attn@v and the softmax denominators).
  - scores computed transposed: sT[k, q] = k_h.T @ q_h, contraction d=64;
    the two heads of a pair sit at partitions 0-63 / 64-127 so their
    QK^T matmuls row-tile the PE array concurrently.
  - softmax denominators collected into one [24, N] tile (cross-partition
    via DMA), one batched DVE reciprocal, broadcast back via DMA with a
    partition-step-0 access pattern, applied to unnormalized outputs.
  - proj: y = a^T.T @ Wp^T + b, a^T already in the right layout.
"""

import numpy as np
import ml_dtypes

B = 16
N = 1025
C = 768
H = 12
D = 64
NCORES = 8
BPC = B // NCORES  # batches per core
NPAD = 1152        # padded key length: 9 * 128
KB = NPAD // 128   # key blocks
IB = C // 128      # input-channel blocks
NQ = 1026          # query extent incl. one even-ing pad column
QCS = [384, 384, 258]   # query chunks covering NQ (all even -> DVE 2x mode)
QCO = [0, 384, 768]
SCALE = D ** -0.5
# minimax cubic p(z) ~ exp(z) on [-0.55, 0.55], factored a3*(z-R)*(z^2+S*z+T);
# exp(y)*EB computed as (p(y/4) * a3*EB^(1/4))^4 in one custom DVE op.
EXP_R = -1.658048394110858
EXP_S = 1.462861309003841
EXP_T = 3.672443055287797
EXP_A3 = 0.1641584267735188
BF16 = ml_dtypes.bfloat16

_cache: dict = {}


def _register_exp4():
    """Register the fused quartic-exp custom DVE op: out = (p(z)*Src1)^4
    with p monic-factored; Src1 carries a3*EB^(1/4)."""
    if "exp4" in _cache:
        return _cache["exp4"]
    import numpy as np
    from concourse import dve_ops
    from concourse.dve_spec import Spec, Src0, Src1, C0, C1, C2
    from concourse.dve_table_gen import dve_ver_for
    from concourse.dve_uop import DveOpSpec

    name = "EXP4_EB_ANT"
    for op in dve_ops.OPS:
        if op.name == name:
            _cache["exp4"] = op
            return op

    m5 = (((Src0 + C1) * Src0) + C2) * (Src0 - C0) * Src1
    p2 = m5 * m5
    body = p2 * p2

    def ref(in0, in1, s0, s1, imm2):
        m5 = ((((in0 + s1) * in0) + imm2) * (in0 - s0) * in1).astype(np.float32)
        p2 = (m5 * m5).astype(np.float32)
        return (p2 * p2).astype(np.float32)

    op = dve_ops.DveOp(name, Spec(body=body, reference=ref), subdim=False,
                       uops_sha={})
    row = dve_ops._CUSTOM_DVE_ROW_BASE + len(dve_ops.OPS)
    assert row < 0x20
    dve_ops.OPS.append(op)
    dve_ops._SUB_OPCODE_FOR_NAME[name] = row
    dve_ops.CUSTOM_DVE_SPECS[name] = op.spec
    ver = dve_ver_for("TRN2")
    spec = DveOpSpec(name=name, opcode=row,
                     uops=dve_ops.lower(op.spec, ver=ver),
                     rd1_en=dve_ops.has_src1(op.spec))
    op.uops_sha[ver] = spec.sha(ver)
    _cache["exp4"] = op
    return op


def _build():
    import concourse.bass as bass
    import concourse.mybir as mybir
    import concourse.tile as tile
    from concourse import bacc

    dt = mybir.dt
    f32 = dt.float32
    bf = dt.bfloat16
    AFT = mybir.ActivationFunctionType

    exp4_op = _register_exp4()
    nc = bacc.Bacc("TRN2", target_bir_lowering=False, debug=False)

    xT_d = nc.declare_dram_parameter("xT", [BPC, C, NPAD], bf, isOutput=False)
    wqk_d = nc.declare_dram_parameter("wqk", [C, 2 * C], bf, isOutput=False)
    wv_d = nc.declare_dram_parameter("wv", [C, C], bf, isOutput=False)
    wp_d = nc.declare_dram_parameter("wp", [C, C], bf, isOutput=False)
    qkb_d = nc.declare_dram_parameter("qkb", [128, 2 * IB], f32, isOutput=False)
    vb_d = nc.declare_dram_parameter("vb", [128, C], f32, isOutput=False)
    pb_d = nc.declare_dram_parameter("pb", [128, C], f32, isOutput=False)
    eb_d = nc.declare_dram_parameter("eb", [H, 3, NPAD, 384], bf, isOutput=False)
    out_d = nc.declare_dram_parameter("out", [BPC, N, C], f32, isOutput=True)

    with tile.TileContext(nc) as tc:
        from contextlib import ExitStack

        ctx = ExitStack()
        with ctx:
            consts = ctx.enter_context(tc.tile_pool(name="consts", bufs=1))
            persist = ctx.enter_context(tc.tile_pool(name="persist", bufs=1))

            # ---- constants ----
            wqk_sb = consts.tile([128, IB, 2 * C], bf)
            nc.sync.dma_start(wqk_sb[:], wqk_d.ap().rearrange("(ib p) o -> p ib o", p=128))
            wp_sb = consts.tile([128, IB, C], bf)
            nc.sync.dma_start(wp_sb[:], wp_d.ap().rearrange("(ib p) o -> p ib o", p=128))
            qkb_sb = consts.tile([128, 2 * IB], f32)
            nc.sync.dma_start(qkb_sb[:], qkb_d.ap())
            vb_sb = consts.tile([128, C], f32)
            nc.sync.dma_start(vb_sb[:], vb_d.ap())
            pb_sb = consts.tile([128, C], f32)
            nc.sync.dma_start(pb_sb[:], pb_d.ap())

            # ---- persistent per-batch tensors ----
            xT_sb = []
            v_sb = []
            a_sb = []
            for b in range(BPC):
                t = persist.tile([128, IB, NPAD], bf, name=f"xT{b}")
                nc.sync.dma_start(t[:], xT_d.ap()[b].rearrange("(ib p) q -> p ib q", p=128))
                xT_sb.append(t)
                v_sb.append(persist.tile([128, KB, H, D + 1], bf, name=f"v{b}"))
                a_sb.append(persist.tile([128, IB, NPAD], bf, name=f"a{b}"))
                # zero the padded query columns so proj sees no garbage
                nc.gpsimd.memset(a_sb[b][:, :, NQ:], 0.0)

            sums_sb = [persist.tile([H, NQ], f32, name=f"sums{b}")
                       for b in range(BPC)]

            # ---- v projection (natural layout, + ones column) ----
            with tc.tile_pool(name="vproj_ps", bufs=2, space="PSUM") as vps, \
                 tc.tile_pool(name="wv_pool", bufs=1) as wvp:
                wv_sb = wvp.tile([128, IB, C], bf)
                nc.sync.dma_start(wv_sb[:], wv_d.ap().rearrange("(ib p) o -> p ib o", p=128))
                for b in range(BPC):
                    nc.gpsimd.memset(v_sb[b][:, :, :, D:], 1.0)
                    for kpb in range(KB):
                        for vc in range(2):
                            ps = vps.tile([128, 384], f32, name="vps")
                            for ib in range(IB):
                                nc.tensor.matmul(
                                    ps[:],
                                    lhsT=xT_sb[b][:, ib, kpb * 128:(kpb + 1) * 128],
                                    rhs=wv_sb[:, ib, vc * 384:(vc + 1) * 384],
                                    start=(ib == 0),
                                    stop=(ib == IB - 1),
                                )
                            nc.vector.tensor_add(
                                out=v_sb[b][:, kpb, 6 * vc:6 * (vc + 1), :D],
                                in0=ps.rearrange("p (h d) -> p h d", d=D),
                                in1=vb_sb[:, vc * 384:(vc + 1) * 384].rearrange(
                                    "p (h d) -> p h d", d=D),
                            )

            # ---- attention over head pairs ----
            attn_ctx = ctx.enter_context(ExitStack())
            qk_pool = attn_ctx.enter_context(tc.tile_pool(name="qk", bufs=4))
            eb_pool = attn_ctx.enter_context(tc.tile_pool(name="ebp", bufs=4))
            ex_pool = attn_ctx.enter_context(tc.tile_pool(name="exp", bufs=3))
            et_pool = attn_ctx.enter_context(tc.tile_pool(name="etmp", bufs=4))
            qk_ps = attn_ctx.enter_context(tc.tile_pool(name="qk_ps", bufs=2, space="PSUM"))
            s_ps = attn_ctx.enter_context(tc.tile_pool(name="s_ps", bufs=4, space="PSUM"))
            o_ps = attn_ctx.enter_context(tc.tile_pool(name="o_ps", bufs=2, space="PSUM"))

            for hp in range(H // 2):
                # q/k projection for this head pair, both batches
                qk2 = []
                for b in range(BPC):
                    t = qk_pool.tile([128, 2, NPAD], bf, name="qk2")
                    nc.gpsimd.memset(t[:, :, N:], 0.0)
                    for sec in range(2):  # 0 = q rows, 1 = k rows
                        ocb = sec * IB + hp
                        for qc in range(3):
                            ps = qk_ps.tile([128, 384], f32, name="qkps")
                            for ib in range(IB):
                                nc.tensor.matmul(
                                    ps[:, :QCS[qc]],
                                    lhsT=wqk_sb[:, ib, sec * C + hp * 128:
                                                sec * C + (hp + 1) * 128],
                                    rhs=xT_sb[b][:, ib, QCO[qc]:QCO[qc] + QCS[qc]],
                                    start=(ib == 0),
                                    stop=(ib == IB - 1),
                                )
                            nc.scalar.activation(
                                out=t[:, sec, QCO[qc]:QCO[qc] + QCS[qc]],
                                in_=ps[:, :QCS[qc]],
                                func=AFT.Identity,
                                bias=qkb_sb[:, ocb:ocb + 1],
                                scale=(SCALE / 4 if sec == 0 else 1.0),
                            )
                    qk2.append(t)

                for qc in range(3):
                    qcs, qco = QCS[qc], QCO[qc]
                    ebt = []
                    for par in range(2):
                        h = 2 * hp + par
                        t = eb_pool.tile([128, KB, 384], bf, name="ebt")
                        nc.sync.dma_start(
                            t[:, :, :qcs],
                            eb_d.ap()[h][qc][:, :qcs].rearrange(
                                "(kb p) q -> p kb q", p=128),
                        )
                        ebt.append(t)
                    for b in range(BPC):
                        ex = [ex_pool.tile([128, KB, 384], bf, name="ex")
                              for _ in range(2)]
                        for kb in range(KB):
                            for par in range(2):
                                p0 = par * 64
                                st = s_ps.tile([128, 384], f32, name="st")
                                nc.tensor.matmul(
                                    st[:, :qcs],
                                    lhsT=qk2[b][p0:p0 + 64, 1, kb * 128:(kb + 1) * 128],
                                    rhs=qk2[b][p0:p0 + 64, 0, qco:qco + qcs],
                                )
                                if par == 0:
                                    nc.vector._custom_dve(
                                        exp4_op,
                                        out=ex[par][:, kb, :qcs],
                                        in0=st[:, :qcs],
                                        in1=ebt[par][:, kb, :qcs],
                                        s0=EXP_R, s1=EXP_S, imm2=EXP_T,
                                    )
                                else:
                                    et = et_pool.tile([128, 384], bf, name="et")
                                    nc.scalar.activation(
                                        out=et[:, :qcs], in_=st[:, :qcs],
                                        func=AFT.Exp, scale=4.0,
                                    )
                                    eng = nc.gpsimd if kb < 6 else nc.vector
                                    eng.tensor_mul(
                                        out=ex[par][:, kb, :qcs],
                                        in0=et[:, :qcs],
                                        in1=ebt[par][:, kb, :qcs],
                                    )
                        for par in range(2):
                            h = 2 * hp + par
                            po = o_ps.tile([D + 1, 384], f32, name="po")
                            for kb in range(KB):
                                nc.tensor.matmul(
                                    po[:, :qcs],
                                    lhsT=v_sb[b][:, kb, h, :],
                                    rhs=ex[par][:, kb, :qcs],
                                    start=(kb == 0),
                                    stop=(kb == KB - 1),
                                )
                            stg = et_pool.tile([65, 384], f32, name="stg")
                            nc.vector.tensor_copy(
                                out=stg[64:65, :qcs], in_=po[D:D + 1, :qcs])
                            nc.sync.dma_start(
                                sums_sb[b][h:h + 1, qco:qco + qcs],
                                stg[64:65, :qcs],
                            )
                            nc.scalar.activation(
                                out=a_sb[b][par * 64:(par + 1) * 64, hp,
                                            qco:qco + qcs],
                                in_=po[:D, :qcs],
                                func=AFT.Copy,
                            )

            attn_ctx.close()

            # ---- per batch: reciprocal, broadcast, normalize, project ----
            recip_dram = nc.dram_tensor("recip_dram", [BPC * H, NQ], bf)
            rb_pool = ctx.enter_context(tc.tile_pool(name="rb", bufs=2))
            with tc.tile_pool(name="y_ps", bufs=4, space="PSUM") as yps, \
                 tc.tile_pool(name="y_sb", bufs=3) as ysb:
                for b in range(BPC):
                    recip_b = persist.tile([H, NQ], bf, name=f"recb{b}")
                    nc.vector.reciprocal_approx_fast(
                        out=sums_sb[b][:], in_=sums_sb[b][:])
                    nc.vector.tensor_copy(out=recip_b[:], in_=sums_sb[b][:])
                    nc.sync.dma_start(
                        recip_dram.ap()[b * H:(b + 1) * H, :], recip_b[:])
                    for h in range(H):
                        p0 = (h % 2) * 64
                        rb = rb_pool.tile([128, NQ], bf, name="rb")
                        src = recip_dram.ap()[b * H + h:b * H + h + 1, :]
                        bcast = bass.AP(
                            tensor=src.tensor,
                            offset=src.offset,
                            ap=[[0, 64]] + list(src.ap[1:]),
                        )
                        nc.sync.dma_start(rb[p0:p0 + 64, :], bcast)
                        nc.vector.tensor_mul(
                            out=a_sb[b][p0:p0 + 64, h // 2, :NQ],
                            in0=a_sb[b][p0:p0 + 64, h // 2, :NQ],
                            in1=rb[p0:p0 + 64, :],
                        )
                    for qb in range(KB):
                        rows = min(128, N - qb * 128)
                        if rows <= 0:
                            continue
                        yt = ysb.tile([128, C], f32, name="yt")
                        for oc2 in range(2):
                            ps = yps.tile([128, 384], f32, name="yps")
                            for ib in range(IB):
                                nc.tensor.matmul(
                                    ps[:],
                                    lhsT=a_sb[b][:, ib, qb * 128:(qb + 1) * 128],
                                    rhs=wp_sb[:, ib, oc2 * 384:(oc2 + 1) * 384],
                                    start=(ib == 0),
                                    stop=(ib == IB - 1),
                                )
                            nc.vector.tensor_add(
                                out=yt[:, oc2 * 384:(oc2 + 1) * 384],
                                in0=ps[:],
                                in1=pb_sb[:, oc2 * 384:(oc2 + 1) * 384],
                            )
                        nc.sync.dma_start(
                            out_d.ap()[b][qb * 128:qb * 128 + rows, :],
                            yt[:rows, :],
                        )

    nc.compile()
    return nc


def _prepare_inputs(x, qkv_weight, q_bias, v_bias, rel_pos_table, proj_weight,
                    proj_bias, rel_pos_index):
    x = np.asarray(x, np.float32)
    qkv_weight = np.asarray(qkv_weight, np.float32)
    q_bias = np.asarray(q_bias, np.float32)
    v_bias = np.asarray(v_bias, np.float32)
    rel_pos_table = np.asarray(rel_pos_table, np.float32)
    proj_weight = np.asarray(proj_weight, np.float32)
    proj_bias = np.asarray(proj_bias, np.float32)
    rel_pos_index = np.asarray(rel_pos_index)

    wqk = np.ascontiguousarray(qkv_weight[:2 * C].T).astype(BF16)
    wv = np.ascontiguousarray(qkv_weight[2 * C:].T).astype(BF16)
    wp = np.ascontiguousarray(proj_weight.T).astype(BF16)

    qkb = np.concatenate([q_bias * np.float32(SCALE / 4), np.zeros(C, np.float32)])
    qkb = np.ascontiguousarray(qkb.reshape(2 * IB, 128).T)  # [128, 12]
    vb = np.ascontiguousarray(np.broadcast_to(v_bias, (128, C)))
    pb = np.ascontiguousarray(np.broadcast_to(proj_bias, (128, C)))

    # exp of transposed rel-pos bias, padded key rows = 0
    bias_qkh = rel_pos_table[rel_pos_index.reshape(-1)].reshape(N, N, H)
    bT = np.zeros((H, N, NQ), np.float64)
    bT[:, :, :N] = bias_qkh.transpose(2, 1, 0)
    ebt = np.zeros((H, NPAD, NQ), BF16)
    for h in range(H):
        if h % 2 == 0:
            ebt[h, :N, :] = (EXP_A3 * np.exp(bT[h] / 4)).astype(BF16)
        else:
            ebt[h, :N, :] = np.exp(bT[h]).astype(BF16)
    ebc = np.zeros((H, 3, NPAD, 384), BF16)
    for qc in range(3):
        ebc[:, qc, :, :QCS[qc]] = ebt[:, :, QCO[qc]:QCO[qc] + QCS[qc]]
    ebt = ebc

    in_maps = []
    for core in range(NCORES):
        xb = x[core * BPC:(core + 1) * BPC]
        xT = np.zeros((BPC, C, NPAD), BF16)
        xT[:, :, :N] = xb.transpose(0, 2, 1).astype(BF16)
        in_maps.append({
            "xT": xT, "wqk": wqk, "wv": wv, "wp": wp,
            "qkb": qkb, "vb": vb, "pb": pb, "eb": ebt,
        })
    return in_maps


def kernel(**inputs) -> np.ndarray:
    from concourse.bass_utils import run_bass_kernel_spmd

    if "nc" not in _cache:
        _cache["nc"] = _build()
    nc = _cache["nc"]

    in_maps = _prepare_inputs(**inputs)
    trace = bool(_cache.get("trace", False))
    res = run_bass_kernel_spmd(nc, in_maps, core_ids=list(range(NCORES)),
                               trace=trace)
    _cache["last_results"] = res
    out = np.concatenate([r["out"] for r in res.results], axis=0)
    return out.astype(np.float32)
